# revision 1
# baseline (speedup 1.0000x reference)
"""Izhikevich spiking scan on 8 Trainium2 cores — linear-scan formulation.

Math (per neuron, no-fire path; validated on host, max |v - v_exact| = 2.9e-7):
    v_t = (4 v^2 + 5 v + 1.4 - r + x_t) * DT,  r_{t-1} = K * v_{t-1}  (r_0 = 0)
  =>  v_t = a_t * v_{t-1} + DT*(1.4 + x_t),  a_t = (5-K)*DT + 4*DT*v_{t-1}
  Substituting V' = v/DT + BETA with BETA = -1.4/(1-ALPHA) absorbs the
  constant:  V'_t = a_t * V'_{t-1} + x_t  (+ a dropped residual
  BETA*(a_t - ALPHA) ~ 1.5e-4, hence the 2.9e-7 v-error).  The
  data-dependent part of a_t is tiny (|4*DT*v| <= 1e-4), so one Picard
  step suffices and v_{t-1} inside a_t can be approximated by its
  dominant term DT*(x_{t-1} - BETA):  a_t = A1SCALE*x_{t-1} + A1BIAS.
  First step is exact: a_0 = 5*DT + 4*DT*C (v_{-1}=C, r_0=0), and the
  t=0 constant goes into data1's first column (COL0FIX).

Layout: host transposes x to [NLOC, T] per core; each DVE scan tile is
[128 neurons, 512 timesteps]; tensor_tensor_scan(mult, add) runs the
recurrence along the free axis with fp32 carry.

Fire handling: v never gets near THRESH=0.3 for sane inputs (max|v| ~ 0.013
for N(0,1) currents). The device emits a per-partition certificate
cert = sum(relu(v - (THRESH - CERT_DELTA))). If any cert > 0, some neuron
came within CERT_DELTA of threshold and the fast path is not trusted:
kernel() falls back to an exact per-step kernel (handles fire/reset
exactly). Otherwise spikes = (v >= THRESH) is exact.
"""

import math
import os
import sys

import numpy as np

if "/opt/trn_rl_repo" not in sys.path:
    sys.path.insert(0, "/opt/trn_rl_repo")

# ---- problem constants ----
T = 512
N = 65536
NCORES = 8
NLOC = N // NCORES          # 8192 neurons per core
P = 128                     # SBUF partitions
NTILE = NLOC // P           # 64 scan tiles of 128 neurons

A = 0.02
B = 0.2
C = -0.065
D = 0.008
DT = 1.0 / T
THRESH = 0.3

K = A * (B - 1.0) * DT
ALPHA = float(np.float32((5.0 - K) * DT))           # steady multiplier
A0COL = float(np.float32(5.0 * DT + 4.0 * DT * C))  # exact t=0 multiplier (r_0=0)
BETA = -1.4 / (1.0 - ALPHA)            # V' = v/DT + BETA  (data1 = x directly)
V0INIT = float(np.float32(C / DT + BETA))
THP = float(np.float32(THRESH / DT + BETA))         # spike threshold, V'-space
CERT_DELTA = 1e-3                                   # in v-units
CERT_TH = float(np.float32(THP - CERT_DELTA / DT))  # near-threshold, V'-space
COL0FIX = 1.4 + BETA * (1.0 - A0COL)                # exact t=0 constant ...
V0INIT2 = float(np.float32(V0INIT + COL0FIX / A0COL))  # ... folded into init
A1SCALE = float(np.float32(4.0 * DT * DT))
A1BIAS = float(np.float32(ALPHA - 4.0 * DT * DT * BETA))

M_DTYPE = os.environ.get("IZI_MDT", "bfloat16")   # spike output container
X_DTYPE = os.environ.get("IZI_XDT", "bfloat16")   # x/a1 on-device dtype
V_DTYPE = os.environ.get("IZI_VDT", "bfloat16")   # V' container dtype
GRP = int(os.environ.get("IZI_GRP", "4"))         # scans per super-tile
FUSED = os.environ.get("IZI_FUSED", "1") == "1"   # one scan instr per group
DEBUG_V = os.environ.get("IZI_DEBUG_V", "0") == "1"  # emit V' for validation

# page-break trick: with multiplier 0 at a page's first column, the carry
# from the previous page dies and V'_{page,0} = x_0 + A0COL*V0INIT2 --
# several neuron pages chain through ONE scan instruction.
CONST_BREAK = float(np.float32(A0COL * V0INIT2))


def _build_scan_nc():
    import concourse.bacc as bacc
    import concourse.mybir as mybir
    from concourse import tile

    fp32 = mybir.dt.float32
    mdt = getattr(mybir.dt, M_DTYPE)
    xdt = getattr(mybir.dt, X_DTYPE)
    vdt = getattr(mybir.dt, V_DTYPE)
    op = mybir.AluOpType
    act = mybir.ActivationFunctionType

    NGRP = NTILE // GRP
    GT = GRP * T

    nc = bacc.Bacc("TRN2", target_bir_lowering=False)
    x_d = nc.dram_tensor("x", [NLOC, T], xdt, kind="ExternalInput")
    m_d = nc.dram_tensor("spk", [NLOC, T], mdt, kind="ExternalOutput")
    cert_d = nc.dram_tensor("cert", [P, NGRP], fp32, kind="ExternalOutput")
    v_d = (nc.dram_tensor("vout", [NLOC, T], vdt, kind="ExternalOutput")
           if DEBUG_V else None)

    def dram_grp(dram, g):
        # rows (g*GRP+j)*P + p  ->  [p, j, t]
        return dram[g * GRP * P:(g + 1) * GRP * P, :].rearrange(
            "(j p) t -> p j t", p=P
        )

    def as3d(tile_ap):
        return tile_ap.rearrange("p (j t) -> p j t", t=T)

    with tile.TileContext(nc) as tc:
        with (
            tc.tile_pool(name="xin", bufs=3) as xin_pool,
            tc.tile_pool(name="vv", bufs=2) as v_pool,
            tc.tile_pool(name="aa", bufs=3) as a_pool,
            tc.tile_pool(name="mm", bufs=2) as m_pool,
            tc.tile_pool(name="junk", bufs=2) as junk_pool,
            tc.tile_pool(name="certp", bufs=1) as cert_pool,
            tc.tile_pool(name="consts", bufs=1) as const_pool,
            tc.tile_pool(name="x0", bufs=1) as x0_pool,
        ):
            cert = cert_pool.tile([P, NGRP], fp32, tag="cert")
            certb = const_pool.tile([P, 1], fp32, tag="cb")
            nc.vector.memset(certb[:], float(-CERT_TH))

            xts = [None] * NGRP
            a1s = [None] * NGRP

            def stage(g):
                """DMA-in + multiplier build for group g (runs ahead)."""
                if g == 0:
                    # group 0 split per tile so the first scan starts after
                    # one 128-row DMA instead of the whole group's
                    xt = [x0_pool.tile([P, T], xdt, tag=f"x0{j}",
                                       name=f"x0t{j}") for j in range(GRP)]
                    a1 = [x0_pool.tile([P, T], xdt, tag=f"a0{j}",
                                       name=f"a0t{j}") for j in range(GRP)]
                    for j in range(GRP):
                        nc.sync.dma_start(
                            out=xt[j][:], in_=x_d[j * P:(j + 1) * P, :])
                        nc.scalar.activation(
                            a1[j][:, 1:T], xt[j][:, 0:T - 1],
                            act.Copy, bias=A1BIAS, scale=A1SCALE,
                        )
                        nc.gpsimd.memset(a1[j][:, 0:1], 0.0)
                        nc.gpsimd.tensor_scalar(
                            xt[j][:, 0:1], xt[j][:, 0:1],
                            CONST_BREAK, None, op.add,
                        )
                    xts[g] = [t[:] for t in xt]
                    a1s[g] = [t[:] for t in a1]
                    return
                xt = xin_pool.tile([P, GT], xdt, tag="xin")
                nc.sync.dma_start(out=as3d(xt[:]), in_=dram_grp(x_d, g))
                a1 = a_pool.tile([P, GT], xdt, tag="a")
                # a1_t = A1SCALE*x_{t-1} + A1BIAS per page; col0 = A0COL
                nc.scalar.activation(
                    as3d(a1[:])[:, :, 1:T], as3d(xt[:])[:, :, 0:T - 1],
                    act.Copy, bias=A1BIAS, scale=A1SCALE,
                )
                if FUSED:
                    # page-break at EVERY page start: zero multiplier kills
                    # the carry, data1 carries the init constant
                    nc.gpsimd.memset(as3d(a1[:])[:, :, 0:1], 0.0)
                    nc.gpsimd.tensor_scalar(
                        as3d(xt[:])[:, :, 0:1],
                        as3d(xt[:])[:, :, 0:1],
                        CONST_BREAK, None, op.add,
                    )
                    xts[g] = [xt[:]]
                    a1s[g] = [a1[:]]
                else:
                    nc.gpsimd.memset(as3d(a1[:])[:, :, 0:1], A0COL)
                    xts[g] = [xt[:, j * T:(j + 1) * T] for j in range(GRP)]
                    a1s[g] = [a1[:, j * T:(j + 1) * T] for j in range(GRP)]

            stage(0)
            stage(1)
            for g in range(NGRP):
                if g + 2 < NGRP:
                    stage(g + 2)
                xt, a1 = xts[g], a1s[g]
                xts[g] = a1s[g] = None

                # the scans: V'_t = a1_t * V'_{t-1} + x_t  (fp32 carry;
                # exact t=0 constant folded into the initial value)
                v = v_pool.tile([P, GT], vdt, tag="v")
                if len(xt) == 1:
                    nc.vector.tensor_tensor_scan(
                        v[:], a1[0], xt[0], 0.0, op.mult, op.add,
                    )
                else:
                    for j in range(GRP):
                        s = slice(j * T, (j + 1) * T)
                        nc.vector.tensor_tensor_scan(
                            v[:, s], a1[j], xt[j], 0.0,
                            op.mult, op.add,
                        )

                # spikes (exact): m = V' >= THP
                mt = m_pool.tile([P, GT], mdt, tag="m")
                nc.vector.tensor_scalar(mt[:], v[:], THP, None, op.is_ge)

                # certificate: sum(relu(V' - CERT_TH)) over the super-tile
                jt = junk_pool.tile([P, GT], vdt, tag="junk")
                nc.scalar.activation(
                    jt[:], v[:], act.Relu,
                    bias=certb[:], scale=1.0,
                    accum_out=cert[:, g:g + 1],
                )
                nc.gpsimd.dma_start(out=dram_grp(m_d, g), in_=as3d(mt[:]))
                if DEBUG_V:
                    nc.sync.dma_start(out=dram_grp(v_d, g), in_=as3d(v[:]))

            nc.sync.dma_start(out=cert_d[:, :], in_=cert[:])
    nc.compile()
    return nc


# ---------------- exact fallback (per-step, handles fire/reset) -------------
_FB = {}


def _build_exact_nc():
    """Exact per-step kernel (the proven baseline formulation)."""
    import concourse.bacc as bacc
    import concourse.mybir as mybir
    from concourse import tile

    fp32 = mybir.dt.float32
    op = mybir.AluOpType
    F = NLOC // P
    TC = 64
    NCHUNK = T // TC

    Kl = A * (B - 1.0) * DT
    beta0 = 320.0 - 25.0 / 16.0 + 1.4
    Thg = THRESH / DT + 320.0
    Rg = C / DT + 320.0
    Rsg = math.sqrt(Rg * Rg - D / (4.0 * DT * DT))
    sigma = 1.0 / (Thg - Rsg)
    C4 = float(np.float32(4.0 * DT * DT / sigma))
    C_R = float(np.float32(-Kl * DT))
    TH_S = float(np.float32(sigma * Thg))
    G0 = float(np.float32(sigma * Rg))
    PRE_SCALE = float(np.float32(sigma))
    PRE_BIAS = float(np.float32(sigma * (beta0 + 320.0 * Kl * DT)))

    nc = bacc.Bacc("TRN2", target_bir_lowering=False)
    x_d = nc.dram_tensor("x", [T, NLOC], fp32, kind="ExternalInput")
    y_d = nc.dram_tensor("spk", [T, NLOC], fp32, kind="ExternalOutput")

    def chunk_view(dram, ci):
        return dram[ci * TC:(ci + 1) * TC, :].rearrange("t (p f) -> p t f", p=P)

    with tile.TileContext(nc) as tc:
        with (
            tc.tile_pool(name="xin", bufs=2) as xin_pool,
            tc.tile_pool(name="pre", bufs=2) as pre_pool,
            tc.tile_pool(name="out", bufs=2) as out_pool,
            tc.tile_pool(name="state", bufs=2) as g_pool,
            tc.tile_pool(name="gp", bufs=2) as gp_pool,
            tc.tile_pool(name="q", bufs=2) as q_pool,
            tc.tile_pool(name="w", bufs=2) as w_pool,
        ):
            pre_tiles = [None] * NCHUNK

            def load_chunk(ci):
                xt = xin_pool.tile([P, TC * F], fp32, tag="xin")
                nc.sync.dma_start(
                    out=xt.rearrange("p (t f) -> p t f", t=TC),
                    in_=chunk_view(x_d, ci),
                )
                pt = pre_pool.tile([P, TC * F], fp32, tag="pre")
                nc.scalar.activation(
                    pt[:], xt[:],
                    mybir.ActivationFunctionType.Copy,
                    bias=PRE_BIAS, scale=PRE_SCALE,
                )
                pre_tiles[ci] = pt

            G = g_pool.tile([P, F], fp32, tag="G")
            nc.vector.memset(G[:], G0)
            load_chunk(0)
            w = None

            for ci in range(NCHUNK):
                if ci + 1 < NCHUNK:
                    load_chunk(ci + 1)
                pre = pre_tiles[ci]
                ot = out_pool.tile([P, TC * F], fp32, tag="out")
                for tt in range(TC):
                    t = ci * TC + tt
                    win = pre[:, 0:F] if t == 0 else w[:]
                    q = q_pool.tile([P, F], fp32, tag="q")
                    nc.vector.tensor_tensor(q[:], G[:], G[:], op.mult)
                    Gp = gp_pool.tile([P, F], fp32, tag="Gp")
                    nc.vector.scalar_tensor_tensor(
                        Gp[:], q[:], C4, win, op.mult, op.add
                    )
                    m = ot[:, tt * F:(tt + 1) * F]
                    nc.vector.tensor_scalar(m, Gp[:], TH_S, None, op.is_ge)
                    if t + 1 < T:
                        if tt + 1 < TC:
                            nxt = pre[:, (tt + 1) * F:(tt + 2) * F]
                        else:
                            nxt = pre_tiles[ci + 1][:, 0:F]
                        w = w_pool.tile([P, F], fp32, tag="w")
                        nc.vector.scalar_tensor_tensor(
                            w[:], Gp[:], C_R, nxt, op.mult, op.add
                        )
                        G = g_pool.tile([P, F], fp32, tag="G")
                        nc.vector.scalar_tensor_tensor(
                            G[:], Gp[:], TH_S, m, op.min, op.subtract
                        )
                pre_tiles[ci] = None
                nc.sync.dma_start(
                    out=chunk_view(y_d, ci),
                    in_=ot.rearrange("p (t f) -> p t f", t=TC),
                )
    nc.compile()
    return nc


def _run_exact(x):
    from concourse.bass_utils import run_bass_kernel_spmd

    if "nc" not in _FB:
        _FB["nc"] = _build_exact_nc()
    nc = _FB["nc"]
    core_ids = list(range(NCORES))
    in_maps = [
        {"x": np.ascontiguousarray(x[:, c * NLOC:(c + 1) * NLOC])}
        for c in core_ids
    ]
    res = run_bass_kernel_spmd(nc, in_maps, core_ids)
    return np.concatenate([res.results[c]["spk"] for c in core_ids], axis=1)


_CACHE = {}


def _core_inputs(x):
    """Per-core input maps: transpose to [NLOC, T], cast to device dtype."""
    if X_DTYPE == "float32":
        xdt = np.float32
    else:
        import ml_dtypes
        xdt = getattr(ml_dtypes, X_DTYPE)
    return [
        {"x": np.ascontiguousarray(x[:, c * NLOC:(c + 1) * NLOC].T.astype(xdt))}
        for c in range(NCORES)
    ]


def kernel(x: np.ndarray) -> np.ndarray:
    from concourse.bass_utils import run_bass_kernel_spmd

    x = np.asarray(x, np.float32)
    assert x.shape == (T, N), x.shape

    if "nc" not in _CACHE:
        _CACHE["nc"] = _build_scan_nc()
    nc = _CACHE["nc"]

    core_ids = list(range(NCORES))
    in_maps = _core_inputs(x)
    res = run_bass_kernel_spmd(nc, in_maps, core_ids)

    # fallback if any neuron came within CERT_DELTA of threshold
    if any(res.results[c]["cert"].max() > 0.0 for c in core_ids):
        return _run_exact(x)

    out = np.empty((T, N), np.float32)
    for c in core_ids:
        out[:, c * NLOC:(c + 1) * NLOC] = \
            res.results[c]["spk"].T.astype(np.float32)
    return out


if __name__ == "__main__":
    xt = np.random.randn(T, N).astype(np.float32)
    y = kernel(xt)
    print("out", y.shape, y.dtype, y.sum())



# revision 9
# speedup vs baseline: 3.1884x; 3.1884x over previous
"""Izhikevich spiking scan on 8 Trainium2 cores — certified zero-spike fast
path + exact per-step fallback.

Mathematical basis (interval bound on the TRUE recurrence, no linearization):
    v_t = (4 v_{t-1}^2 + 5 v_{t-1} + 1.4 - r_{t-1} + x_t) * DT,  DT = 1/512
    r_t = K * v_t with K = A(B-1)DT = -3.125e-5   (while no neuron has fired)
  Suppose max_t |x_t| <= X <= 150 for a neuron. Then |v| stays below the
  fixed point  v̄ = DT(4v̄² + 5v̄ + 1.4 + |K|v̄ + X):  for X = 124,
  v̄ ≈ 0.2478 < THRESH = 0.3 (and |v_0| = 0.065 < v̄, r_0 = 0), so
  v_t < 0.3 for ALL t and the spike output is IDENTICALLY ZERO.
  A first spike would require v_t >= 0.3, impossible.  Hence:

      max_t |x_t| <= 124 for every neuron  ==>  output == zeros.   (*)

Device certificate: the host casts y = |x| to float8_e4m3fn (1 byte/elem;
monotone encoding for non-negative values, NaN = 0x7F).  Byte >= 0x70 iff
y >= 128 or NaN;  byte <= 0x6F  ==>  |x| < 124 (since |x| >= 124 rounds to
fp8 >= 128).  So if EVERY byte of the cast tensor is < 0x70, (*) applies
and the output is exactly zeros.  Otherwise kernel() falls back to the
exact per-step kernel (handles fire/reset exactly).

Device kernel: bytes are packed little-endian into uint16 words
w = b_lo + 256*b_hi (each b <= 0x7F: fp8 of |x| has no sign bit).  Tests:
    hi: w >= 0x7000              (b_hi >= 0x70)
    lo: (w & 0x0070) >= 0x0070   (b_lo >= 0x70; bits 6..4 all set)
The reduction over all words is split across three engines so each stays
under the DMA shadow (~12.6 us for 4 MiB/core):
  - Activation: relu(w - 28671.5) / relu(m - 111.5) with sum-accum
    (exact on integers; zero iff no flagged byte in the slice)
  - DVE:     max-accum (TENSOR_SCALAR_CACHE_REDUCE)
  - GpSimd:  max-accum
with one 4x DVE pass building m = w & 0x0070.
Host checks: act sums == 0, DVE/Pool maxes below 28672 / 112.
"""

import math
import os
import sys

import numpy as np

if "/opt/trn_rl_repo" not in sys.path:
    sys.path.insert(0, "/opt/trn_rl_repo")

# ---- problem constants ----
T = 512
N = 65536
NCORES = 8
NLOC = N // NCORES          # 8192 neurons per core
P = 128                     # SBUF partitions

A = 0.02
B = 0.2
C = -0.065
D = 0.008
DT = 1.0 / T
THRESH = 0.3

BYTES_PC = NLOC * T         # 4 MiB of fp8 bytes per core
U16_PC = BYTES_PC // 2      # 2_097_152 uint16 words per core
FREE = U16_PC // P          # 16384 words per partition

NCHUNK = int(os.environ.get("IZI_NCHUNK", "4"))
CH = FREE // NCHUNK

# engine split of each chunk's CH words: n_act to Activation (relu-accum),
# n_dve to DVE max-accum, remainder to PE (bf16 flags + ones-matmul sums).
N_ACT = int(os.environ.get("IZI_NACT", "1536"))
N_DVE = int(os.environ.get("IZI_NDVE", "0"))
PE_BLK = 512                # matmul moving-block width (one PSUM bank)

TH_HI = 0x7000              # w >= TH_HI  <=> high byte >= 0x70
TH_LO = 0x0070              # (w & 0x0070) == 0x0070 <=> low byte >= 0x70


def _build_cert_nc():
    import concourse.bacc as bacc
    import concourse.mybir as mybir
    from concourse import tile

    fp32 = mybir.dt.float32
    u16 = mybir.dt.uint16
    op = mybir.AluOpType
    act = mybir.ActivationFunctionType

    from concourse.bass import MemorySpace

    fp16 = mybir.dt.float16
    bf16 = mybir.dt.bfloat16

    n_act = N_ACT
    n_dve = N_DVE
    n_pe = CH - n_act - n_dve
    assert n_pe % PE_BLK == 0, (n_pe, PE_BLK)
    s_act = slice(0, n_act)
    s_dve = slice(n_act, n_act + n_dve)
    s_pe = slice(n_act + n_dve, CH)
    n_mm = NCHUNK * 2 * (n_pe // PE_BLK)   # total matmuls

    nc = bacc.Bacc("TRN2", target_bir_lowering=False)
    x_d = nc.dram_tensor("x", [P, FREE], u16, kind="ExternalInput")
    # certificates: act sums (hi, lo) per chunk; dve maxes; PE flag sums
    certa_d = nc.dram_tensor("certa", [P, NCHUNK * 2], fp32,
                             kind="ExternalOutput")
    if n_dve:
        certd_d = nc.dram_tensor("certd", [P, NCHUNK * 2], fp32,
                                 kind="ExternalOutput")
    certm_d = nc.dram_tensor("certm", [1, PE_BLK], fp32,
                             kind="ExternalOutput")

    with tile.TileContext(nc) as tc:
        with (
            tc.tile_pool(name="xin", bufs=3) as xp,
            tc.tile_pool(name="mm", bufs=2) as mp,
            tc.tile_pool(name="ff", bufs=3) as fp,
            tc.tile_pool(name="jj", bufs=1) as jp,
            tc.tile_pool(name="cc", bufs=1) as cp,
            tc.tile_pool(name="ps", bufs=1, space=MemorySpace.PSUM) as pp,
        ):
            certa = cp.tile([P, NCHUNK * 2], fp32, tag="ca")
            certd = cp.tile([P, NCHUNK * 2], fp32, tag="cd") if n_dve else None
            bias_hi = cp.tile([P, 1], fp32, tag="bh")
            nc.vector.memset(bias_hi[:], -(TH_HI - 0.5))
            bias_lo = cp.tile([P, 1], fp32, tag="bl")
            nc.vector.memset(bias_lo[:], -(TH_LO - 0.5))
            ones = cp.tile([P, 1], bf16, tag="ones")
            nc.vector.memset(ones[:], 1.0)
            # junk elementwise outputs for the accum instructions
            ja = jp.tile([P, max(n_act, 1)], fp16, tag="ja")
            jd = jp.tile([P, max(n_dve, 1)], u16, tag="jd")
            psum = pp.tile([1, PE_BLK], fp32, tag="ps")
            csum = cp.tile([1, PE_BLK], fp32, tag="cs")

            mm_i = 0
            for c in range(NCHUNK):
                w = xp.tile([P, CH], u16, tag="w")
                nc.sync.dma_start(out=w[:], in_=x_d[:, c * CH:(c + 1) * CH])
                # m = w & 0x0070  (DVE 4x)
                m = mp.tile([P, CH], u16, tag="m")
                nc.vector.tensor_scalar(m[:], w[:], TH_LO, None,
                                        op.bitwise_and, op.bypass)
                co = c * 2
                # Activation share: sum(relu(val + bias)) — 0 iff clean
                if n_act:
                    nc.scalar.activation(ja[:], w[:, s_act], act.Relu,
                                         bias=bias_hi[:], scale=1.0,
                                         accum_out=certa[:, co:co + 1])
                    nc.scalar.activation(ja[:], m[:, s_act], act.Relu,
                                         bias=bias_lo[:], scale=1.0,
                                         accum_out=certa[:, co + 1:co + 2])
                # DVE share: max-accum
                if n_dve:
                    nc.vector.tensor_scalar(jd[:], w[:, s_dve], 1.0, None,
                                            op.mult, op.max,
                                            accum_out=certd[:, co:co + 1])
                    nc.vector.tensor_scalar(jd[:], m[:, s_dve], 1.0, None,
                                            op.mult, op.max,
                                            accum_out=certd[:, co + 1:co + 2])
                # PE share: bf16 danger flags (DVE 4x) summed by matmul
                if n_pe:
                    fh = fp.tile([P, n_pe], bf16, tag="fh")
                    nc.vector.tensor_scalar(fh[:], w[:, s_pe], float(TH_HI),
                                            None, op.is_ge, op.bypass)
                    fl = fp.tile([P, n_pe], bf16, tag="fl")
                    nc.vector.tensor_scalar(fl[:], m[:, s_pe], float(TH_LO),
                                            None, op.is_ge, op.bypass)
                    for f in (fh, fl):
                        for b in range(n_pe // PE_BLK):
                            nc.tensor.matmul(
                                psum[:],
                                ones[:],
                                f[:, b * PE_BLK:(b + 1) * PE_BLK],
                                start=(mm_i == 0),
                                stop=(mm_i == n_mm - 1),
                            )
                            mm_i += 1

            nc.any.tensor_copy(csum[:], psum[:])
            nc.sync.dma_start(out=certa_d[:, :], in_=certa[:])
            if n_dve:
                nc.sync.dma_start(out=certd_d[:, :], in_=certd[:])
            nc.sync.dma_start(out=certm_d[:, :], in_=csum[:])
    nc.compile()
    return nc


def _cert_clean(r):
    """True iff the certificates prove every fp8 byte < 0x70."""
    certa = np.asarray(r["certa"], np.float32)
    certm = np.asarray(r["certm"], np.float32)
    if not (np.isfinite(certa).all() and np.isfinite(certm).all()):
        return False
    ok = bool((certa == 0.0).all()) and bool((certm == 0.0).all())
    if N_DVE and ok:
        certd = np.asarray(r["certd"], np.float32).reshape(P, NCHUNK, 2)
        ok = (np.isfinite(certd).all()
              and bool((certd[:, :, 0] < TH_HI).all())
              and bool((certd[:, :, 1] < TH_LO).all()))
    return ok


# ---------------- exact fallback (per-step, handles fire/reset) -------------
_FB = {}


def _build_exact_nc():
    """Exact per-step kernel (the proven baseline formulation)."""
    import concourse.bacc as bacc
    import concourse.mybir as mybir
    from concourse import tile

    fp32 = mybir.dt.float32
    op = mybir.AluOpType
    F = NLOC // P
    TC = 64
    NCHUNKF = T // TC

    Kl = A * (B - 1.0) * DT
    beta0 = 320.0 - 25.0 / 16.0 + 1.4
    Thg = THRESH / DT + 320.0
    Rg = C / DT + 320.0
    Rsg = math.sqrt(Rg * Rg - D / (4.0 * DT * DT))
    sigma = 1.0 / (Thg - Rsg)
    C4 = float(np.float32(4.0 * DT * DT / sigma))
    C_R = float(np.float32(-Kl * DT))
    TH_S = float(np.float32(sigma * Thg))
    G0 = float(np.float32(sigma * Rg))
    PRE_SCALE = float(np.float32(sigma))
    PRE_BIAS = float(np.float32(sigma * (beta0 + 320.0 * Kl * DT)))

    nc = bacc.Bacc("TRN2", target_bir_lowering=False)
    x_d = nc.dram_tensor("x", [T, NLOC], fp32, kind="ExternalInput")
    y_d = nc.dram_tensor("spk", [T, NLOC], fp32, kind="ExternalOutput")

    def chunk_view(dram, ci):
        return dram[ci * TC:(ci + 1) * TC, :].rearrange("t (p f) -> p t f", p=P)

    with tile.TileContext(nc) as tc:
        with (
            tc.tile_pool(name="xin", bufs=2) as xin_pool,
            tc.tile_pool(name="pre", bufs=2) as pre_pool,
            tc.tile_pool(name="out", bufs=2) as out_pool,
            tc.tile_pool(name="state", bufs=2) as g_pool,
            tc.tile_pool(name="gp", bufs=2) as gp_pool,
            tc.tile_pool(name="q", bufs=2) as q_pool,
            tc.tile_pool(name="w", bufs=2) as w_pool,
        ):
            pre_tiles = [None] * NCHUNKF

            def load_chunk(ci):
                xt = xin_pool.tile([P, TC * F], fp32, tag="xin")
                nc.sync.dma_start(
                    out=xt.rearrange("p (t f) -> p t f", t=TC),
                    in_=chunk_view(x_d, ci),
                )
                pt = pre_pool.tile([P, TC * F], fp32, tag="pre")
                nc.scalar.activation(
                    pt[:], xt[:],
                    mybir.ActivationFunctionType.Copy,
                    bias=PRE_BIAS, scale=PRE_SCALE,
                )
                pre_tiles[ci] = pt

            G = g_pool.tile([P, F], fp32, tag="G")
            nc.vector.memset(G[:], G0)
            load_chunk(0)
            w = None

            for ci in range(NCHUNKF):
                if ci + 1 < NCHUNKF:
                    load_chunk(ci + 1)
                pre = pre_tiles[ci]
                ot = out_pool.tile([P, TC * F], fp32, tag="out")
                for tt in range(TC):
                    t = ci * TC + tt
                    win = pre[:, 0:F] if t == 0 else w[:]
                    q = q_pool.tile([P, F], fp32, tag="q")
                    nc.vector.tensor_tensor(q[:], G[:], G[:], op.mult)
                    Gp = gp_pool.tile([P, F], fp32, tag="Gp")
                    nc.vector.scalar_tensor_tensor(
                        Gp[:], q[:], C4, win, op.mult, op.add
                    )
                    mm = ot[:, tt * F:(tt + 1) * F]
                    nc.vector.tensor_scalar(mm, Gp[:], TH_S, None, op.is_ge)
                    if t + 1 < T:
                        if tt + 1 < TC:
                            nxt = pre[:, (tt + 1) * F:(tt + 2) * F]
                        else:
                            nxt = pre_tiles[ci + 1][:, 0:F]
                        w = w_pool.tile([P, F], fp32, tag="w")
                        nc.vector.scalar_tensor_tensor(
                            w[:], Gp[:], C_R, nxt, op.mult, op.add
                        )
                        G = g_pool.tile([P, F], fp32, tag="G")
                        nc.vector.scalar_tensor_tensor(
                            G[:], Gp[:], TH_S, mm, op.min, op.subtract
                        )
                pre_tiles[ci] = None
                nc.sync.dma_start(
                    out=chunk_view(y_d, ci),
                    in_=ot.rearrange("p (t f) -> p t f", t=TC),
                )
    nc.compile()
    return nc


def _run_exact(x):
    from concourse.bass_utils import run_bass_kernel_spmd

    if "nc" not in _FB:
        _FB["nc"] = _build_exact_nc()
    nc = _FB["nc"]
    core_ids = list(range(NCORES))
    in_maps = [
        {"x": np.ascontiguousarray(x[:, c * NLOC:(c + 1) * NLOC])}
        for c in core_ids
    ]
    res = run_bass_kernel_spmd(nc, in_maps, core_ids)
    return np.concatenate([res.results[c]["spk"] for c in core_ids], axis=1)


_CACHE = {}


def _core_inputs(x):
    """Per-core input maps: |x| as fp8e4m3fn bytes, packed as uint16 words.

    Core c owns columns [c*NLOC, (c+1)*NLOC); any byte order works for the
    certificate, so the fp8 bytes are just viewed flat as [P, FREE] uint16.
    """
    import ml_dtypes

    y = np.abs(x).astype(ml_dtypes.float8_e4m3fn)   # [T, N] 1 byte/elem
    maps = []
    for c in range(NCORES):
        yc = np.ascontiguousarray(y[:, c * NLOC:(c + 1) * NLOC])
        maps.append({"x": yc.view(np.uint16).reshape(P, FREE)})
    return maps


def kernel(x: np.ndarray) -> np.ndarray:
    from concourse.bass_utils import run_bass_kernel_spmd

    x = np.asarray(x, np.float32)
    assert x.shape == (T, N), x.shape

    if "nc" not in _CACHE:
        _CACHE["nc"] = _build_cert_nc()
    nc = _CACHE["nc"]

    core_ids = list(range(NCORES))
    in_maps = _core_inputs(x)
    res = run_bass_kernel_spmd(nc, in_maps, core_ids)

    if all(_cert_clean(res.results[c]) for c in core_ids):
        return np.zeros((T, N), np.float32)
    return _run_exact(x)


if __name__ == "__main__":
    xt = np.random.randn(T, N).astype(np.float32)
    y = kernel(xt)
    print("out", y.shape, y.dtype, y.sum())
